# revision 12
# baseline (speedup 1.0000x reference)
"""Trainium2 Bass kernel for nn_GATBlock (GAT encoder/decoder VAE block).

Sharding: nodes partitioned across 8 cores by dst range (2500/core).
Edges bucketed by dst owner, sorted by dst, blocked per 128-dst group.
Weights replicated. Aggregation via one-hot selection matmuls on PE,
row gathers via gpsimd dma_gather. 3 launches with host concat between
(graph-global exchange points: h/as1 table, z table).
"""
import os
import sys

for _p in ("/opt/trn_rl_repo", "/root/.axon_site/_ro/trn_rl_repo"):
    if os.path.isdir(_p) and _p not in sys.path:
        sys.path.append(_p)

import numpy as np

import concourse.bass as bass
import concourse.mybir as mybir
import concourse.tile as tile
from concourse import bacc
from concourse.bass_utils import run_bass_kernel_spmd

F32 = mybir.dt.float32
F16 = mybir.dt.float16
I16 = mybir.dt.int16

N, IN, HID, LAT, E = 20000, 3000, 512, 32, 320000
NCORE = 8
SH = N // NCORE            # 2500 nodes per core
SHP = 2512                 # padded shard rows (multiple of 16, >= SH+1 for sentinel)
NROWS = NCORE * SHP        # padded global table rows
AUGW = 640                 # fp16 cols: 512 h | as1_hi | as1_lo | pad   (1280B rows)
ZW = 128                   # fp16 cols: 32 z | pad                      (256B rows)
P = 128
NB = (SH + P - 1) // P     # dst blocks per core (20; last has 68 rows)
SENT = SH                  # local pad row used as sentinel (as1 = -60000)
EXP_SHIFT = -4.0           # exp(e + EXP_SHIFT): softmax-invariant, keeps fp16 S safe

KT = [(k, min(P, IN - k)) for k in range(0, IN, P)]        # k tiles over IN
MT = [(m, min(P, SH - m)) for m in range(0, SH, P)]        # m tiles over shard rows
HKT = HID // P                                             # 4 k tiles over HID


def _pad16(a):
    return np.float16(a)


def _pack_col(v):
    """[SH] -> [128, NB]: element [p, b] = v[b*128+p] (zero-padded)."""
    vp = np.zeros(NB * P, v.dtype)
    vp[:SH] = v
    return np.ascontiguousarray(vp.reshape(NB, P).T)


def _wrap_idx(idx):
    """[L] int -> [128, L//16] int16 (16-part wrap replicated for 8 Q7 cores)."""
    L = len(idx)
    assert L % 16 == 0
    w = np.asarray(idx, np.int16).reshape(L // 16, 16).T  # [16, L/16]
    return np.tile(w, (8, 1)).copy()


def _preprocess_edges(edge_index):
    """Bucket edges by dst owner, sort by dst, pad per 128-dst block.

    Returns per-core dicts with wrapped gather indices (into the padded
    global table), dstlocal (0..127 within block, fp16), and the shared
    per-block chunk capacities cb[b] (count of 128-edge chunks).
    """
    src = np.asarray(edge_index[0], np.int64)
    dst = np.asarray(edge_index[1], np.int64)
    owner = dst // SH
    cores = []
    counts = np.zeros((NCORE, NB), np.int64)
    per_core_blocks = []
    for c in range(NCORE):
        m = owner == c
        s_c, d_c = src[m], dst[m] - c * SH
        order = np.argsort(d_c, kind="stable")
        s_c, d_c = s_c[order], d_c[order]
        blk = d_c // P
        blocks = []
        for b in range(NB):
            bm = blk == b
            bs = s_c[bm]
            bd = d_c[bm] - b * P
            counts[c, b] = len(bs)
            blocks.append((bs, bd))
        per_core_blocks.append(blocks)
    cb = [max(1, int(np.max(np.ceil(counts[:, b] / P)))) for b in range(NB)]
    for c in range(NCORE):
        idx_parts, dl_parts = [], []
        for b in range(NB):
            bs, bd = per_core_blocks[c][b]
            cap = cb[b] * P
            gi = (bs // SH) * SHP + (bs % SH)          # padded-global row
            gi = np.concatenate([gi, np.full(cap - len(gi), SENT, np.int64)])
            bd = np.concatenate([bd, np.zeros(cap - len(bd), np.int64)])
            idx_parts.append(gi)
            # device layout: [p, j] = value of edge j*128+p
            dl_parts.append(bd.reshape(cb[b], P).T)
        idx = np.concatenate(idx_parts)
        dl = np.concatenate(dl_parts, axis=1)          # [128, sum(cb)]
        cores.append(dict(idx=_wrap_idx(idx), dl=_pad16(dl), dlint=dl.astype(np.int64)))
    return cores, cb


def _expand_ad1(ad1_core, dlint, cb):
    """[SH] f32 + dstlocal ints -> per-edge ad1 [128, sum(cb)] f32."""
    out = np.empty_like(dlint, dtype=np.float32)
    ad1p = np.concatenate([ad1_core, np.zeros(NB * P - SH, np.float32)])
    coff = 0
    for b in range(NB):
        out[:, coff : coff + cb[b]] = ad1p[b * P + dlint[:, coff : coff + cb[b]]]
        coff += cb[b]
    return out


# ---------------------------------------------------------------- L1 ----
def build_l1():
    nc = bacc.Bacc("TRN2", target_bir_lowering=False, debug=False, num_devices=NCORE)
    xs = nc.declare_dram_parameter("xs", [IN, SH], F32, isOutput=False)
    w1 = nc.declare_dram_parameter("w1", [IN, HID], F32, isOutput=False)
    vc = nc.declare_dram_parameter("vc", [IN, 2], F32, isOutput=False)
    lt = nc.declare_dram_parameter("lt", [P, 24], F32, isOutput=False)
    aug = nc.declare_dram_parameter("aug", [SHP, AUGW], F16, isOutput=True)
    ad1 = nc.declare_dram_parameter("ad1", [SH, 1], F32, isOutput=True)
    theta = nc.declare_dram_parameter("theta", [P, 24], F32, isOutput=True)

    nkt = len(KT)
    with tile.TileContext(nc) as tc:
        with (
            tc.tile_pool(name="const", bufs=1) as cpool,
            tc.tile_pool(name="sb", bufs=3) as sb,
            tc.tile_pool(name="ep", bufs=2) as ep,
            tc.tile_pool(name="ps", bufs=2, space="PSUM") as ps,
            tc.tile_pool(name="ps2", bufs=2, space="PSUM") as ps2,
        ):
            w1t = cpool.tile([P, nkt, HID], F16)
            vct = cpool.tile([P, nkt, 2], F16)
            for t, (k0, kw) in enumerate(KT):
                nc.gpsimd.dma_start(w1t[:kw, t, :], w1[k0 : k0 + kw, :])
                nc.gpsimd.dma_start(vct[:kw, t, :], vc[k0 : k0 + kw, :])

            # theta = exp(log_theta), host-packed [128, 24]
            lt_sb = cpool.tile([P, 24], F32)
            th_sb = cpool.tile([P, 24], F32)
            nc.sync.dma_start(lt_sb[:], lt[:, :])
            nc.scalar.activation(th_sb[:], lt_sb[:], mybir.ActivationFunctionType.Exp)
            nc.sync.dma_start(theta[:, :], th_sb[:])

            # sentinel pad rows: as1 = -60000 so padded edges get ex = 0
            sent = cpool.tile([SHP - SH, 2], F16)
            nc.vector.memset(sent[:], -60000.0)
            nc.sync.dma_start(aug[SH:SHP, HID : HID + 2], sent[:])

            for m0, mw in MT:
                xt = sb.tile([P, nkt, P], F16, tag="xt")
                for t, (k0, kw) in enumerate(KT):
                    nc.gpsimd.dma_start(
                        xt[:kw, t, :mw], xs[k0 : k0 + kw, m0 : m0 + mw]
                    )
                acc = ps.tile([P, HID], F32)
                acc2 = ps2.tile([P, 8], F32)
                for t, (k0, kw) in enumerate(KT):
                    nc.tensor.matmul(
                        acc[:mw, :], xt[:kw, t, :mw], w1t[:kw, t, :],
                        start=(t == 0), stop=(t == nkt - 1),
                    )
                    nc.tensor.matmul(
                        acc2[:mw, :2], xt[:kw, t, :mw], vct[:kw, t, :],
                        start=(t == 0), stop=(t == nkt - 1),
                    )
                pack = ep.tile([P, HID + 2], F16, tag="pack")
                asb = ep.tile([P, 2], F32, tag="asb")
                nc.vector.tensor_copy(pack[:mw, :HID], acc[:mw, :])
                # as1 hi/lo split (fp16 pair reconstructs f32 as1)
                nc.vector.tensor_copy(pack[:mw, HID : HID + 1], acc2[:mw, 0:1])
                nc.vector.tensor_tensor(
                    out=pack[:mw, HID + 1 : HID + 2],
                    in0=acc2[:mw, 0:1],
                    in1=pack[:mw, HID : HID + 1],
                    op=mybir.AluOpType.subtract,
                )
                nc.vector.tensor_copy(asb[:mw, 1:2], acc2[:mw, 1:2])
                nc.sync.dma_start(aug[m0 : m0 + mw, : HID + 2], pack[:mw, :])
                nc.sync.dma_start(ad1[m0 : m0 + mw, :], asb[:mw, 1:2])
    nc.compile()
    return nc


# ---------------------------------------------------------------- L2 ----
def build_l2(cb):
    nc = bacc.Bacc("TRN2", target_bir_lowering=False, debug=False, num_devices=NCORE)
    sumcb = sum(cb)
    maxcb = max(cb)
    aug = nc.declare_dram_parameter("aug", [NROWS, AUGW], F16, isOutput=False)
    idxd = nc.declare_dram_parameter("idx", [P, sumcb * 8], I16, isOutput=False)
    dld = nc.declare_dram_parameter("dl", [P, sumcb], F16, isOutput=False)
    ad1d = nc.declare_dram_parameter("ad1", [P, sumcb], F32, isOutput=False)
    epsd = nc.declare_dram_parameter("eps", [SH, LAT], F32, isOutput=False)
    w2d = nc.declare_dram_parameter("w2", [HID, LAT], F32, isOutput=False)
    wmd = nc.declare_dram_parameter("wm", [LAT, LAT], F32, isOutput=False)
    wvd = nc.declare_dram_parameter("wv", [LAT, LAT], F32, isOutput=False)
    bmd = nc.declare_dram_parameter("bm", [1, LAT], F32, isOutput=False)
    bvd = nc.declare_dram_parameter("bv", [1, LAT], F32, isOutput=False)
    mean_o = nc.declare_dram_parameter("mean", [SH, LAT], F32, isOutput=True)
    lv_o = nc.declare_dram_parameter("lv", [SH, LAT], F32, isOutput=True)
    z_o = nc.declare_dram_parameter("z", [SH, LAT], F32, isOutput=True)
    zp_o = nc.declare_dram_parameter("zp", [SHP, ZW], F16, isOutput=True)
    ex_o = nc.declare_dram_parameter("ex", [P, sumcb], F32, isOutput=True)
    rd_o = nc.declare_dram_parameter("rd", [P, NB], F32, isOutput=True)

    iota = nc.inline_tensor(
        np.tile(np.arange(P, dtype=np.float16), (P, maxcb)), name="iota"
    )  # [128, maxcb*128], [p, j*128+i] = i
    ones_col = nc.inline_tensor(np.ones((P, 1), np.float16), name="onescol")
    ident = nc.inline_tensor(np.eye(P, dtype=np.float16), name="ident")
    ones_row = nc.inline_tensor(np.ones((1, P), np.float16), name="onesrow")

    AF = mybir.ActivationFunctionType
    OP = mybir.AluOpType
    with tile.TileContext(nc) as tc:
        with (
            tc.tile_pool(name="const", bufs=1) as cpool,
            tc.tile_pool(name="gp", bufs=3) as gp,
            tc.tile_pool(name="sp", bufs=2) as sp,
            tc.tile_pool(name="wk", bufs=2) as wk,
            tc.tile_pool(name="pagg", bufs=2, space="PSUM") as pagg,
            tc.tile_pool(name="psm", bufs=1, space="PSUM") as psm,
        ):
            iota_t = cpool.tile([P, maxcb * P], F16)
            nc.sync.dma_start(iota_t[:], iota[:, :])
            ones_t = cpool.tile([P, 1], F16)
            nc.sync.dma_start(ones_t[:], ones_col[:, :])
            id_t = cpool.tile([P, P], F16)
            nc.sync.dma_start(id_t[:], ident[:, :])
            or_t = cpool.tile([1, P], F16)
            nc.sync.dma_start(or_t[:], ones_row[:, :])
            idx_t = cpool.tile([P, sumcb * 8], I16)
            nc.sync.dma_start(idx_t[:], idxd[:, :])
            dl_t = cpool.tile([P, sumcb], F16)
            nc.sync.dma_start(dl_t[:], dld[:, :])
            ad1_t = cpool.tile([P, sumcb], F32)
            nc.sync.dma_start(ad1_t[:], ad1d[:, :])
            w2_t = cpool.tile([P, HKT, LAT], F16)
            for k in range(HKT):
                nc.gpsimd.dma_start(w2_t[:, k, :], w2d[k * P : (k + 1) * P, :])
            wm_t = cpool.tile([LAT, LAT], F16)
            nc.gpsimd.dma_start(wm_t[:], wmd[:, :])
            wv_t = cpool.tile([LAT, LAT], F16)
            nc.gpsimd.dma_start(wv_t[:], wvd[:, :])
            # broadcast biases to all partitions via rank-1 matmul
            bm_sb = cpool.tile([1, LAT], F16)
            nc.gpsimd.dma_start(bm_sb[:], bmd[:, :])
            bv_sb = cpool.tile([1, LAT], F16)
            nc.gpsimd.dma_start(bv_sb[:], bvd[:, :])
            bb_ps = psm.tile([P, 2 * LAT], F32, tag="bbps")
            nc.tensor.matmul(bb_ps[:, :LAT], or_t[:1, :], bm_sb[:1, :], start=True, stop=True)
            nc.tensor.matmul(bb_ps[:, LAT:], or_t[:1, :], bv_sb[:1, :], start=True, stop=True)
            bb_t = cpool.tile([P, 2 * LAT], F32)
            nc.vector.tensor_copy(bb_t[:], bb_ps[:])
            shift_t = cpool.tile([P, 1], F32)
            nc.vector.memset(shift_t[:], EXP_SHIFT)

            coff = 0
            for b in range(NB):
                m0 = b * P
                mw = min(P, SH - m0)
                cbb = cb[b]
                ni = cbb * P
                g = gp.tile([P, cbb, AUGW], F16, tag="g")
                for j0 in range(0, cbb, 8):
                    nj = min(8, cbb - j0)
                    nc.gpsimd.dma_gather(
                        out_ap=g[:, j0 : j0 + nj, :],
                        in_ap=aug[:, :],
                        idxs_ap=idx_t[:, (coff + j0) * 8 : (coff + j0 + nj) * 8],
                        num_idxs=nj * P,
                        num_idxs_reg=nj * P,
                        elem_size=AUGW,
                    )
                # e = leaky_relu(as1 + ad1_dst), ex = exp(e - 4)
                exf = wk.tile([P, cbb], F32, tag="exf")
                nc.vector.tensor_tensor(
                    out=exf[:], in0=g[:, :, HID], in1=g[:, :, HID + 1], op=OP.add
                )
                nc.vector.tensor_tensor(
                    out=exf[:], in0=exf[:], in1=ad1_t[:, coff : coff + cbb], op=OP.add
                )
                lr = wk.tile([P, cbb], F32, tag="lr")
                nc.vector.tensor_scalar(
                    out=lr[:], in0=exf[:], scalar1=0.2, scalar2=None, op0=OP.mult
                )
                nc.vector.tensor_tensor(out=exf[:], in0=exf[:], in1=lr[:], op=OP.max)
                nc.scalar.activation(exf[:], exf[:], AF.Exp, bias=shift_t[:, 0:1])
                nc.sync.dma_start(ex_o[:, coff : coff + cbb], exf[:])
                exh = wk.tile([P, cbb], F16, tag="exh")
                nc.vector.tensor_copy(exh[:], exf[:])
                # S^T[p, j, i] = ex[p,j] * (dl[p,j] == i)
                s3 = sp.tile([P, cbb, P], F16, tag="s3")
                nc.vector.tensor_tensor(
                    out=s3[:],
                    in0=iota_t[:, : cbb * P].rearrange("p (j i) -> p j i", i=P),
                    in1=dl_t[:, coff : coff + cbb].to_broadcast([P, cbb, P]),
                    op=OP.is_equal,
                )
                nc.vector.tensor_tensor(
                    out=s3[:], in0=s3[:],
                    in1=exh[:].to_broadcast([P, cbb, P]),
                    op=OP.mult,
                )
                agg = pagg.tile([P, HID], F32, tag="agg")
                den = psm.tile([P, 8], F32, tag="den")
                for j in range(cbb):
                    nc.tensor.matmul(
                        agg[:], s3[:, j, :], g[:, j, :HID],
                        start=(j == 0), stop=(j == cbb - 1),
                    )
                    nc.tensor.matmul(
                        den[:, 0:1], s3[:, j, :], ones_t[:, :],
                        start=(j == 0), stop=(j == cbb - 1),
                    )
                rden = wk.tile([P, 1], F32, tag="rden")
                nc.vector.tensor_scalar(
                    out=rden[:], in0=den[:, 0:1], scalar1=1e-16, scalar2=None, op0=OP.add
                )
                nc.vector.reciprocal(rden[:], rden[:])
                nc.sync.dma_start(rd_o[:, b : b + 1], rden[:])
                h1 = wk.tile([P, HID], F32, tag="h1")
                nc.vector.tensor_scalar(
                    out=h1[:], in0=agg[:], scalar1=rden[:, 0:1], scalar2=None, op0=OP.mult
                )
                # elu: r = relu(h1); h1e = exp(h1 - r) + r - 1
                relu = wk.tile([P, HID], F32, tag="relu")
                nc.scalar.activation(relu[:], h1[:], AF.Relu)
                nc.vector.tensor_tensor(out=h1[:], in0=h1[:], in1=relu[:], op=OP.subtract)
                nc.scalar.activation(h1[:], h1[:], AF.Exp)
                nc.vector.tensor_tensor(out=h1[:], in0=h1[:], in1=relu[:], op=OP.add)
                h1e = wk.tile([P, HID], F16, tag="h1e")
                nc.vector.tensor_scalar(
                    out=h1e[:], in0=h1[:], scalar1=-1.0, scalar2=None, op0=OP.add
                )
                # hidden = h1e @ W2s  (transpose h1e per 128-col tile)
                htp = psm.tile([P, HKT, P], F16, tag="htp")
                h1t = wk.tile([P, HKT, P], F16, tag="h1t")
                for k in range(HKT):
                    nc.tensor.transpose(htp[:, k, :], h1e[:, k * P : (k + 1) * P], id_t[:])
                    nc.vector.tensor_copy(h1t[:, k, :], htp[:, k, :])
                hid = psm.tile([P, LAT], F32, tag="hid")
                for k in range(HKT):
                    nc.tensor.matmul(
                        hid[:mw, :], h1t[:, k, :mw], w2_t[:, k, :],
                        start=(k == 0), stop=(k == HKT - 1),
                    )
                hsb = wk.tile([P, LAT], F16, tag="hsb")
                nc.vector.tensor_copy(hsb[:mw, :], hid[:mw, :])
                htp2 = psm.tile([LAT, P], F16, tag="htp2")
                nc.tensor.transpose(htp2[:, :mw], hsb[:mw, :], id_t[:mw, :mw])
                hts = wk.tile([LAT, P], F16, tag="hts")
                nc.vector.tensor_copy(hts[:, :mw], htp2[:, :mw])
                mv = psm.tile([P, 2 * LAT], F32, tag="mv")
                nc.tensor.matmul(mv[:mw, :LAT], hts[:, :mw], wm_t[:, :], start=True, stop=True)
                nc.tensor.matmul(mv[:mw, LAT:], hts[:, :mw], wv_t[:, :], start=True, stop=True)
                mean = wk.tile([P, LAT], F32, tag="mean")
                nc.vector.tensor_tensor(out=mean[:mw, :], in0=mv[:mw, :LAT], in1=bb_t[:mw, :LAT], op=OP.add)
                nc.sync.dma_start(mean_o[m0 : m0 + mw, :], mean[:mw, :])
                lv = wk.tile([P, LAT], F32, tag="lv")
                nc.vector.tensor_tensor(out=lv[:mw, :], in0=mv[:mw, LAT:], in1=bb_t[:mw, LAT:], op=OP.add)
                nc.vector.tensor_scalar(
                    out=lv[:mw, :], in0=lv[:mw, :], scalar1=10.0, scalar2=-10.0,
                    op0=OP.min, op1=OP.max,
                )
                nc.sync.dma_start(lv_o[m0 : m0 + mw, :], lv[:mw, :])
                std = wk.tile([P, LAT], F32, tag="std")
                nc.scalar.activation(std[:mw, :], lv[:mw, :], AF.Exp, scale=0.5)
                nc.vector.tensor_scalar(
                    out=std[:mw, :], in0=std[:mw, :], scalar1=1e-8, scalar2=None, op0=OP.add
                )
                nc.scalar.activation(std[:mw, :], std[:mw, :], AF.Sqrt)
                epst = wk.tile([P, LAT], F32, tag="epst")
                nc.sync.dma_start(epst[:mw, :], epsd[m0 : m0 + mw, :])
                z = wk.tile([P, LAT], F32, tag="z")
                nc.vector.tensor_tensor(out=z[:mw, :], in0=epst[:mw, :], in1=std[:mw, :], op=OP.mult)
                nc.vector.tensor_tensor(out=z[:mw, :], in0=z[:mw, :], in1=mean[:mw, :], op=OP.add)
                nc.sync.dma_start(z_o[m0 : m0 + mw, :], z[:mw, :])
                zh = wk.tile([P, LAT], F16, tag="zh")
                nc.vector.tensor_copy(zh[:mw, :], z[:mw, :])
                nc.sync.dma_start(zp_o[m0 : m0 + mw, :LAT], zh[:mw, :])
                coff += cbb
    nc.compile()
    return nc


# ---------------------------------------------------------------- L3 ----
def build_l3(cb):
    nc = bacc.Bacc("TRN2", target_bir_lowering=False, debug=False, num_devices=NCORE)
    sumcb = sum(cb)
    maxcb = max(cb)
    zp = nc.declare_dram_parameter("zp", [NROWS, ZW], F16, isOutput=False)
    idxd = nc.declare_dram_parameter("idx", [P, sumcb * 8], I16, isOutput=False)
    dld = nc.declare_dram_parameter("dl", [P, sumcb], F16, isOutput=False)
    exd = nc.declare_dram_parameter("ex", [P, sumcb], F32, isOutput=False)
    rdd = nc.declare_dram_parameter("rd", [P, NB], F32, isOutput=False)
    w2td = nc.declare_dram_parameter("w2t", [LAT, HID], F32, isOutput=False)
    w1td = nc.declare_dram_parameter("w1t", [HID, IN], F32, isOutput=False)
    mu_o = nc.declare_dram_parameter("mu", [SH, IN], F32, isOutput=True)

    iota = nc.inline_tensor(
        np.tile(np.arange(P, dtype=np.float16), (P, maxcb)), name="iota"
    )
    ident = nc.inline_tensor(np.eye(P, dtype=np.float16), name="ident")

    NT = [(n, min(512, IN - n)) for n in range(0, IN, 512)]  # 6 recon col tiles
    AF = mybir.ActivationFunctionType
    OP = mybir.AluOpType
    with tile.TileContext(nc) as tc:
        with (
            tc.tile_pool(name="const", bufs=1) as cpool,
            tc.tile_pool(name="gp", bufs=3) as gp,
            tc.tile_pool(name="sp", bufs=2) as sp,
            tc.tile_pool(name="wk", bufs=2) as wk,
            tc.tile_pool(name="mu", bufs=2) as mupool,
            tc.tile_pool(name="pr", bufs=3, space="PSUM") as pr,
            tc.tile_pool(name="psm", bufs=1, space="PSUM") as psm,
        ):
            iota_t = cpool.tile([P, maxcb * P], F16)
            nc.sync.dma_start(iota_t[:], iota[:, :])
            id_t = cpool.tile([P, P], F16)
            nc.sync.dma_start(id_t[:], ident[:, :])
            idx_t = cpool.tile([P, sumcb * 8], I16)
            nc.sync.dma_start(idx_t[:], idxd[:, :])
            dl_t = cpool.tile([P, sumcb], F16)
            nc.sync.dma_start(dl_t[:], dld[:, :])
            ex_t = cpool.tile([P, sumcb], F16)
            nc.gpsimd.dma_start(ex_t[:], exd[:, :])  # cast f32 -> f16
            rd_t = cpool.tile([P, NB], F32)
            nc.sync.dma_start(rd_t[:], rdd[:, :])
            w2t_t = cpool.tile([LAT, HID], F16)
            nc.gpsimd.dma_start(w2t_t[:], w2td[:, :])
            w1t_t = cpool.tile([P, HKT, IN], F16)
            for k in range(HKT):
                nc.gpsimd.dma_start(w1t_t[:, k, :], w1td[k * P : (k + 1) * P, :])

            coff = 0
            for b in range(NB):
                m0 = b * P
                mw = min(P, SH - m0)
                cbb = cb[b]
                ni = cbb * P
                g = gp.tile([P, cbb, ZW], F16, tag="g")
                for j0 in range(0, cbb, 8):
                    nj = min(8, cbb - j0)
                    nc.gpsimd.dma_gather(
                        out_ap=g[:, j0 : j0 + nj, :],
                        in_ap=zp[:, :],
                        idxs_ap=idx_t[:, (coff + j0) * 8 : (coff + j0 + nj) * 8],
                        num_idxs=nj * P,
                        num_idxs_reg=nj * P,
                        elem_size=ZW,
                    )
                s3 = sp.tile([P, cbb, P], F16, tag="s3")
                nc.vector.tensor_tensor(
                    out=s3[:],
                    in0=iota_t[:, : cbb * P].rearrange("p (j i) -> p j i", i=P),
                    in1=dl_t[:, coff : coff + cbb].to_broadcast([P, cbb, P]),
                    op=OP.is_equal,
                )
                nc.vector.tensor_tensor(
                    out=s3[:], in0=s3[:],
                    in1=ex_t[:, coff : coff + cbb].to_broadcast([P, cbb, P]),
                    op=OP.mult,
                )
                aggz = psm.tile([P, LAT], F32, tag="aggz")
                for j in range(cbb):
                    nc.tensor.matmul(
                        aggz[:], s3[:, j, :], g[:, j, :LAT],
                        start=(j == 0), stop=(j == cbb - 1),
                    )
                az = wk.tile([P, LAT], F16, tag="az")
                nc.vector.tensor_scalar(
                    out=az[:], in0=aggz[:], scalar1=rd_t[:, b : b + 1], scalar2=None,
                    op0=OP.mult,
                )
                azp = psm.tile([LAT, P], F16, tag="azp")
                nc.tensor.transpose(azp[:], az[:], id_t[:])
                azt = wk.tile([LAT, P], F16, tag="azt")
                nc.vector.tensor_copy(azt[:], azp[:])
                h3p = psm.tile([P, HID], F32, tag="h3p")
                nc.tensor.matmul(h3p[:], azt[:, :], w2t_t[:, :], start=True, stop=True)
                # elu
                h3f = wk.tile([P, HID], F32, tag="h3f")
                relu = wk.tile([P, HID], F32, tag="relu")
                nc.scalar.activation(relu[:], h3p[:], AF.Relu)
                nc.vector.tensor_tensor(out=h3f[:], in0=h3p[:], in1=relu[:], op=OP.subtract)
                nc.scalar.activation(h3f[:], h3f[:], AF.Exp)
                nc.vector.tensor_tensor(out=h3f[:], in0=h3f[:], in1=relu[:], op=OP.add)
                h3 = wk.tile([P, HID], F16, tag="h3")
                nc.vector.tensor_scalar(
                    out=h3[:], in0=h3f[:], scalar1=-1.0, scalar2=None, op0=OP.add
                )
                htp = psm.tile([P, HKT, P], F16, tag="htp")
                h3t = wk.tile([P, HKT, P], F16, tag="h3t")
                for k in range(HKT):
                    nc.tensor.transpose(htp[:, k, :], h3[:, k * P : (k + 1) * P], id_t[:])
                    nc.vector.tensor_copy(h3t[:, k, :], htp[:, k, :])
                mu_sb = mupool.tile([P, IN], F32, tag="musb")
                for n0, nw in NT:
                    rp = pr.tile([P, 512], F32, tag="rp")
                    for k in range(HKT):
                        nc.tensor.matmul(
                            rp[:mw, :nw], h3t[:, k, :mw], w1t_t[:, k, n0 : n0 + nw],
                            start=(k == 0), stop=(k == HKT - 1),
                        )
                    sp1 = wk.tile([P, 512], F32, tag="sp1")
                    nc.scalar.activation(sp1[:mw, :nw], rp[:mw, :nw], AF.Exp, scale=-1.0)
                    nc.vector.tensor_scalar(
                        out=sp1[:mw, :nw], in0=sp1[:mw, :nw], scalar1=1.0, scalar2=None, op0=OP.add
                    )
                    nc.scalar.activation(sp1[:mw, :nw], sp1[:mw, :nw], AF.Ln)
                    nc.vector.tensor_tensor(
                        out=mu_sb[:mw, n0 : n0 + nw], in0=sp1[:mw, :nw], in1=rp[:mw, :nw], op=OP.add
                    )
                nc.sync.dma_start(mu_o[m0 : m0 + mw, :], mu_sb[:mw, :])
                coff += cbb
    nc.compile()
    return nc


# ------------------------------------------------------------- driver ----
TRACE = bool(os.environ.get("GAT_TRACE"))


def _install_ntff_hook():
    """Provide antenv.axon_hooks (absent in this image) so trace=True works."""
    import types

    try:
        from antenv.axon_hooks import get_axon_ntff_profile_hook  # noqa: F401
        return
    except ImportError:
        pass
    import antenv
    from trn_agent_boot.trn_boot import _ntff_profile_via_ctypes

    hook = _ntff_profile_via_ctypes("/opt/axon/libaxon_pjrt.so")
    m = types.ModuleType("antenv.axon_hooks")
    m._hook = hook
    m.set_axon_ntff_profile_hook = lambda h: setattr(m, "_hook", h)
    m.get_axon_ntff_profile_hook = lambda: m._hook
    sys.modules["antenv.axon_hooks"] = m
    antenv.axon_hooks = m
    import concourse.bass_utils as _bu

    _bu.upload_artifacts = lambda tmpdir: f"local:{tmpdir}"


if TRACE:
    _install_ntff_hook()
LAST_EXEC_NS = []
LAST_TRACES = []
_cache = {}


def _run(prog, ins, core_ids):
    r = run_bass_kernel_spmd(prog, ins, core_ids, trace=TRACE)
    if r.exec_time_ns is not None:
        LAST_EXEC_NS.append(r.exec_time_ns)
    if r.instructions_and_trace is not None:
        LAST_TRACES.append(r.instructions_and_trace[1])
    return r.results


def _programs(cb):
    key = tuple(cb)
    if key not in _cache:
        _cache[key] = (build_l1(), build_l2(cb), build_l3(cb))
    return _cache[key]


def kernel(**inputs):
    x = np.asarray(inputs["x"], np.float32)
    edge_index = np.asarray(inputs["edge_index"])
    eps = np.asarray(inputs["eps"], np.float32)
    W1s = np.asarray(inputs["W1s"], np.float32)
    W1d = np.asarray(inputs["W1d"], np.float32)
    a1s = np.asarray(inputs["a1s"], np.float32)
    a1d = np.asarray(inputs["a1d"], np.float32)
    W2s = np.asarray(inputs["W2s"], np.float32)
    Wm = np.asarray(inputs["Wm"], np.float32)
    bm = np.asarray(inputs["bm"], np.float32)
    Wv = np.asarray(inputs["Wv"], np.float32)
    bv = np.asarray(inputs["bv"], np.float32)
    log_theta = np.asarray(inputs["log_theta"], np.float32)

    LAST_EXEC_NS.clear()
    LAST_TRACES.clear()
    cores, cb = _preprocess_edges(edge_index)
    l1, l2, l3 = _programs(cb)
    core_ids = list(range(NCORE))

    ltp = np.zeros(24 * P, np.float32)
    ltp[:IN] = log_theta
    ltp = np.ascontiguousarray(ltp.reshape(24, P).T)
    v1s = (W1s.astype(np.float64) @ a1s.astype(np.float64)).astype(np.float32)
    v1d = (W1d.astype(np.float64) @ a1d.astype(np.float64)).astype(np.float32)
    vc = np.stack([v1s, v1d], axis=1)  # [IN, 2]
    xT = np.ascontiguousarray(x.T)     # [IN, N]

    in1 = [
        dict(
            xs=np.ascontiguousarray(xT[:, c * SH : (c + 1) * SH]),
            w1=W1s, vc=vc, lt=ltp,
        )
        for c in core_ids
    ]
    import time as _time
    _t = _time.time()
    r1 = _run(l1, in1, core_ids)
    print(f"[kernel] L1 done {_time.time()-_t:.1f}s", flush=True)
    aug_full = np.concatenate([r1[c]["aug"] for c in core_ids], axis=0)
    theta = r1[0]["theta"].T.reshape(-1)[:IN].copy()

    in2 = [
        dict(
            aug=aug_full,
            idx=cores[c]["idx"],
            dl=cores[c]["dl"],
            ad1=_expand_ad1(r1[c]["ad1"][:, 0], cores[c]["dlint"], cb),
            eps=np.ascontiguousarray(eps[c * SH : (c + 1) * SH]),
            w2=W2s, wm=Wm, wv=Wv, bm=bm[None, :], bv=bv[None, :],
        )
        for c in core_ids
    ]
    _t = _time.time()
    r2 = _run(l2, in2, core_ids)
    print(f"[kernel] L2 done {_time.time()-_t:.1f}s", flush=True)
    zp_full = np.concatenate([r2[c]["zp"] for c in core_ids], axis=0)

    W2sT = np.ascontiguousarray(W2s.T)
    W1sT = np.ascontiguousarray(W1s.T)
    in3 = [
        dict(
            zp=zp_full,
            idx=cores[c]["idx"],
            dl=cores[c]["dl"],
            ex=r2[c]["ex"],
            rd=r2[c]["rd"],
            w2t=W2sT, w1t=W1sT,
        )
        for c in core_ids
    ]
    _t = _time.time()
    r3 = _run(l3, in3, core_ids)
    print(f"[kernel] L3 done {_time.time()-_t:.1f}s", flush=True)

    mean = np.concatenate([r2[c]["mean"] for c in core_ids], axis=0)
    log_var = np.concatenate([r2[c]["lv"] for c in core_ids], axis=0)
    z = np.concatenate([r2[c]["z"] for c in core_ids], axis=0)
    mu = np.concatenate([r3[c]["mu"] for c in core_ids], axis=0)
    return mean, log_var, mu, theta, z
